# revision 12
# baseline (speedup 1.0000x reference)
"""Trainium2 Bass kernel for the e3nn-style 5x5x5 SAME conv (dense_cnn).

Strategy
--------
Data-parallel: 8 shards = 2 batches x 4 x-slabs of 12 output planes each.
Each core gets a zero/halo-padded, channel-first input slab and produces
[64, 12, 48, 48].

Sparsity-aware tap packing (see baseline docstring): 57 of 125 taps are
nonzero; the conv is a sum of 39 PSUM-accumulated K=128 matmul "entries"
per output tile, using a dup-plane layout (partitions 0-63 = ch at z,
64-127 = ch at z+1, z stored de-interleaved as (parity, half)).

Mixed fp8/bf16 precision (this version): output energy is dominated by a
few entries (the center self-connection entry alone is ~69%).  The top-7
entries by energy run in bf16 as before; the remaining 32 are quantized
to fp8 e4m3 (TRN clip +-240, weights pre-scaled by a power of 2) and run
as 16 MatmulPerfMode.DoubleRow instructions, each fusing TWO entries
(K=256) at ~2x bf16 column throughput.  The DR rhs addresses the two
entries' plane slices with a hand-built 4-dim access pattern
[part][pair][y][z].  All matmuls accumulate into the same fp32 PSUM
group; the weight scale is divided out on the host after gather.  bf16
plane copies load lazily (only planes some bf16 entry reads).
Simulated end-to-end rel err: 1.82e-2 (gate 2e-2).
"""

import math

import numpy as np
import ml_dtypes
import bass_rust

import concourse.bass as bass
import concourse.mybir as mybir
from concourse import bacc, bass_utils
from concourse.tile import TileContext

MUL = 16
NB = 4
R = 2.5

N_CORES = 8
PX, PY, PZ = 16, 52, 52          # padded per-core input slab (x, y, z)
OX, OY, OZ = 12, 48, 48          # per-core output region
HPL = PZ // 2                    # 26 z-halves per parity block
PLANE2 = PY * PZ                 # 2704 cols per dup plane (y major, z=(par,h))
OPLANE = OY * OZ                 # 2304 outputs per x-plane
YB = 3                           # y-blocks of 16 rows -> N = 16*24 = 384
YBS = OY // YB
N_BF16 = 7                       # entries kept in bf16 (top by energy)


def _entries():
    """(tx, ty, c) matmul list; tap tz(s, q) = c + s - q."""
    out = []
    for tx in range(5):
        for ty in range(5):
            d2 = (tx - 2) ** 2 + (ty - 2) ** 2
            if d2 in (0, 1):
                cs = (0, 2, 4)
            elif d2 in (2, 4):
                cs = (1, 3)
            elif d2 == 5:
                cs = (2,)
            else:
                cs = ()
            for c in cs:
                out.append((tx, ty, c))
    return out


ENTRIES = _entries()
N_W = len(ENTRIES)               # 39


def _build_k(w000, w011, w101, w110, sc0, sc1):
    """Numpy port of the reference kernel build. Returns [5,5,5,64,64]."""
    s = 2
    c = np.arange(-s, s + 1.0)
    lat = np.stack(np.meshgrid(c, c, c, indexing='ij'), axis=-1)
    norm = np.linalg.norm(lat, axis=-1)
    safe = np.where(norm == 0.0, 1.0, norm)
    nvec = np.where(norm[..., None] > 0.0, lat / safe[..., None], 0.0)
    sh1 = np.sqrt(3.0) * nvec
    values = np.linspace(0.0, R, NB + 2)[1:-1]
    step = R / (NB + 1)
    d = (norm[..., None] - values) / step
    dd = np.clip(d, -1.0 + 1e-9, 1.0 - 1e-9)
    emb = np.where(np.abs(d) < 1.0,
                   1.14136 * np.e ** 2 * np.exp(-1.0 / (1.0 - dd ** 2)), 0.0)
    nlat = 125.0

    r000 = np.einsum('xyzb,buw->xyzuw', emb, w000) / nlat
    r011 = np.einsum('xyzb,buw->xyzuw', emb, w011) / nlat
    r101 = np.einsum('xyzb,buw->xyzuw', emb, w101) / nlat
    r110 = np.einsum('xyzb,buw->xyzuw', emb, w110) / nlat
    eye3 = np.eye(3)
    k00 = r000
    k01 = np.einsum('xyzuw,xyzk->xyzuwk', r011, sh1).reshape(5, 5, 5, MUL, 3 * MUL)
    k11 = np.einsum('xyzuw,ik->xyzuiwk', r101, eye3).reshape(5, 5, 5, 3 * MUL, 3 * MUL)
    k10 = np.einsum('xyzuw,xyzi->xyzuiw', r110, sh1).reshape(5, 5, 5, 3 * MUL, MUL) / np.sqrt(3.0)
    top = np.concatenate([k00, k01], axis=-1)
    bot = np.concatenate([k10, k11], axis=-1)
    k = np.concatenate([top, bot], axis=-2)

    lin00 = sc0 / np.sqrt(float(MUL))
    lin11 = np.einsum('uw,ik->uiwk', sc1 / np.sqrt(float(MUL)), eye3).reshape(3 * MUL, 3 * MUL)
    z16 = np.zeros((MUL, 3 * MUL))
    lin = np.concatenate([
        np.concatenate([lin00, z16], axis=1),
        np.concatenate([z16.T, lin11], axis=1)], axis=0)
    k[2, 2, 2] = lin
    return k


def _config(k):
    """Pick bf16 entries (top N_BF16 by output-energy) and pair the rest
    for DoubleRow, pairing within each tx group (same SBUF plane tile).

    Returns (bf16_by_tx, pairs_by_tx, wscale):
      bf16_by_tx[tx] = [entry_idx, ...]
      pairs_by_tx[tx] = [(eA, eB|None), ...]  ordered so sliceB offset >
                        sliceA offset.
    """
    Etap = np.sum(k ** 2, axis=(3, 4))
    Eent = np.zeros(N_W)
    for ei, (tx, ty, c) in enumerate(ENTRIES):
        for s in range(2):
            for q in range(2):
                tz = c + s - q
                if 0 <= tz <= 4:
                    Eent[ei] += Etap[tx, ty, tz] / 2.0
    bf16_set = set(np.argsort(-Eent)[:N_BF16].tolist())

    def soff(ei):
        _, ty, c = ENTRIES[ei]
        return ty * PZ + (c & 1) * HPL + (c >> 1)

    bf16_by_tx, pairs_by_tx = [], []
    max8 = 0.0
    for tx in range(5):
        idxs = [ei for ei, e in enumerate(ENTRIES) if e[0] == tx]
        bf16_by_tx.append([ei for ei in idxs if ei in bf16_set])
        fp8 = sorted((ei for ei in idxs if ei not in bf16_set), key=soff)
        pr = []
        for j in range(0, len(fp8) - 1, 2):
            pr.append((fp8[j], fp8[j + 1]))
        if len(fp8) % 2:
            pr.append((fp8[-1], None))
        pairs_by_tx.append(pr)
        for ei in fp8:
            tx_, ty, c = ENTRIES[ei]
            for s in range(2):
                for q in range(2):
                    tz = c + s - q
                    if 0 <= tz <= 4:
                        max8 = max(max8, np.abs(k[tx_, ty, tz]).max())
    wscale = 2.0 ** math.floor(math.log2(224.0 / max8))
    return bf16_by_tx, pairs_by_tx, wscale


def _entry_block(k, ei, scale):
    """[128,128] f32 block for entry ei: B[s*64+ci, q*64+co] =
    k[tx,ty,c+s-q][ci,co] * scale."""
    tx, ty, c = ENTRIES[ei]
    B = np.zeros((128, 128), np.float32)
    for s in range(2):
        for q in range(2):
            tz = c + s - q
            if 0 <= tz <= 4:
                B[s * 64:(s + 1) * 64, q * 64:(q + 1) * 64] = \
                    k[tx, ty, tz] * scale
    return B


def _pack_weights(k, cfg):
    bf16_by_tx, pairs_by_tx, wscale = cfg
    w16 = np.concatenate(
        [_entry_block(k, ei, wscale)
         for tx in range(5) for ei in bf16_by_tx[tx]], axis=1)
    blocks8 = []
    for tx in range(5):
        for (eA, eB) in pairs_by_tx[tx]:
            blocks8.append(_entry_block(k, eA, wscale))
            blocks8.append(_entry_block(k, eB, wscale)
                           if eB is not None else np.zeros((128, 128), np.float32))
    w8 = np.concatenate(blocks8, axis=1)
    w16 = np.ascontiguousarray(w16).astype(ml_dtypes.bfloat16)
    w8 = np.clip(np.ascontiguousarray(w8), -240, 240) \
        .astype(ml_dtypes.float8_e4m3fn)
    return w16, w8


_NC = None
_NC_KEY = None


def _get_nc(cfg):
    global _NC, _NC_KEY
    key = (tuple(map(tuple, cfg[0])), tuple(map(tuple, cfg[1])))
    if _NC is None or _NC_KEY != key:
        _NC = _build_nc(cfg)
        _NC_KEY = key
    return _NC


def _build_nc(cfg):
    bf16_by_tx, pairs_by_tx, _ = cfg
    n16 = sum(len(b) for b in bf16_by_tx)
    npr = sum(len(p) for p in pairs_by_tx)
    NI = n16 + npr                   # accumulation-group length per y-block

    nc = bacc.Bacc("TRN2", target_bir_lowering=False)
    f32 = mybir.dt.float32
    bf16 = mybir.dt.bfloat16
    fp8 = mybir.dt.float8e4

    xin8 = nc.dram_tensor("xin8", [128, PX * PLANE2], fp8, kind="ExternalInput")
    xin16 = nc.dram_tensor("xin16", [128, PX * PLANE2], bf16,
                           kind="ExternalInput")
    wts16 = nc.dram_tensor("wts16", [128, max(n16, 1) * 128], bf16,
                           kind="ExternalInput")
    wts8 = nc.dram_tensor("wts8", [128, npr * 256], fp8, kind="ExternalInput")
    # output staged/DMAed as bf16 (halves output DMA; ~1e-3 extra rel err,
    # negligible in quadrature); host converts back to f32 and unscales
    yout = nc.dram_tensor("yout", [64, OX * OPLANE], bf16,
                          kind="ExternalOutput")

    with TileContext(nc) as tc:
        with tc.tile_pool(name="wpool", bufs=1) as wpool, \
             tc.tile_pool(name="xpool8", bufs=8) as xpool8, \
             tc.tile_pool(name="xpool16", bufs=8) as xpool16, \
             tc.tile_pool(name="opool", bufs=2) as opool, \
             tc.tile_pool(name="ppool", bufs=6, space="PSUM") as ppool:

            planes8, planes16 = {}, {}

            def _load_plane(px, pool, dt, src, tag):
                # dup plane tile: col (y, par, h); partitions 0-63 hold ch
                # at z=2h+par, 64-127 hold z+1.  One DMA per plane: each
                # dynamic-DMA issue costs ~650ns on the Sync queue, and the
                # 3-y-block entry group needs the whole plane anyway.
                pt = pool.tile([128, PLANE2], dt, tag=tag, name=tag)
                base = px * PLANE2
                nc.sync.dma_start(out=pt[:, :],
                                  in_=src[:, base:base + PLANE2])
                return pt

            def get_plane8(px):
                if px not in planes8:
                    planes8[px] = _load_plane(px, xpool8, fp8, xin8,
                                              "plane8")
                return planes8[px]

            def get_plane16(px):
                # lazy: only planes actually read by a bf16 entry load
                if px not in planes16:
                    planes16[px] = _load_plane(px, xpool16, bf16, xin16,
                                               "plane16")
                return planes16[px]

            # per-tx weight chunks, interleaved with plane loads so the
            # first matmuls' dependencies land first
            n16_tx = [len(b) for b in bf16_by_tx]
            npr_tx = [len(p) for p in pairs_by_tx]
            off16 = [sum(n16_tx[:t]) for t in range(5)]
            off8 = [sum(npr_tx[:t]) for t in range(5)]
            wt16_chunks = [None] * 5
            wt8_chunks = [None] * 5

            def load_chunk(txc):
                if npr_tx[txc]:
                    w8c = wpool.tile([128, npr_tx[txc] * 256], fp8,
                                     tag="wt8", bufs=5, name="wt8")
                    o = off8[txc] * 256
                    nc.sync.dma_start(out=w8c[:, :],
                                      in_=wts8[:, o:o + npr_tx[txc] * 256])
                    wt8_chunks[txc] = w8c
                if n16_tx[txc]:
                    w16c = wpool.tile([128, n16_tx[txc] * 128], bf16,
                                      tag="wt16", bufs=5, name="wt16")
                    o = off16[txc] * 128
                    nc.sync.dma_start(out=w16c[:, :],
                                      in_=wts16[:, o:o + n16_tx[txc] * 128])
                    wt16_chunks[txc] = w16c

            # prefetch in first-consumption order: the xo=0 instruction
            # stream needs (wt chunk tx, fp8 plane tx, bf16 plane tx if
            # that tx has bf16 entries) for tx = 0..4
            load_chunk(0)
            get_plane8(0)
            if n16_tx[0]:
                get_plane16(0)
            for txc in range(1, 5):
                load_chunk(txc)
                get_plane8(txc)
                if n16_tx[txc]:
                    get_plane16(txc)
            get_plane8(5)

            # warm-up: dummy matmuls ramp the PE clock (0.65->2.4 GHz after
            # ~3us busy) while the first input DMAs are in flight
            warm = wpool.tile([128, 128], bf16, tag="warm", bufs=1,
                              name="warm")
            nc.any.memset(warm[:, :], 0)
            wps = ppool.tile([128, 512], f32, tag="wps", bufs=1, name="wps")
            for _ in range(16):
                nc.tensor.matmul(wps[:, :128], warm[:, :], warm[:, :],
                                 start=True, stop=True)

            def zoff_of(ei):
                _, _, c = ENTRIES[ei]
                return (c & 1) * HPL + (c >> 1)

            def bf16_mm(ps, txc, jj, ei, ptv16, y0, start, stop):
                _, ty, _ = ENTRIES[ei]
                zo = zoff_of(ei)
                lhsT = wt16_chunks[txc][:, jj * 128:(jj + 1) * 128]
                rhs = ptv16[:, y0 + ty:y0 + ty + YBS, zo:zo + OZ // 2]
                nc.tensor.matmul(ps[:, :], lhsT, rhs, start=start, stop=stop)

            def dr_mm(ps, txc, pp, pair, pt8, ptv8, y0, start, stop):
                eA, eB = pair
                _, tyA, _ = ENTRIES[eA]
                zA = zoff_of(eA)
                sA = ptv8[:, y0 + tyA:y0 + tyA + YBS, zA:zA + OZ // 2]
                if eB is None:
                    delta = 1
                else:
                    _, tyB, _ = ENTRIES[eB]
                    delta = (tyB - tyA) * PZ + (zoff_of(eB) - zA)
                rhs = sA.unsqueeze(1)
                pat = [list(p) for p in rhs.ap]
                pat[1] = [delta, 2]
                rhs.ap = bass_rust.VecI64Pair(pat)
                lhsT = wt8_chunks[txc][:, :].rearrange(
                    "c (p j m) -> c p j m", j=2, m=128)[:, pp, :, :]
                nc.tensor.matmul(ps[:, :], lhsT, rhs,
                                 perf_mode=mybir.MatmulPerfMode.DoubleRow,
                                 start=start, stop=stop)

            def copy_out(ostv, psv, y0, q):
                # alternate DVE / ACT so the two per-bank copies drain in
                # parallel instead of serializing on Vector
                dst = ostv[:, y0:y0 + YBS, q:OZ:2]
                src = psv[q * 64:(q + 1) * 64, :, :]
                if q == 0:
                    nc.vector.tensor_copy(dst, src)
                else:
                    nc.scalar.activation(
                        dst, src, mybir.ActivationFunctionType.Copy)

            for xo in range(OX):
                ostage = opool.tile([64, OPLANE], bf16, name="ostage")
                ostv = ostage.rearrange("c (y z) -> c y z", z=OZ)
                last = xo == OX - 1
                if not last:
                    pss = [ppool.tile([128, 512], f32, name="ps")
                           for _ in range(YB)]
                    i = 0
                    for txc in range(5):
                        pt8 = get_plane8(xo + txc)
                        ptv8 = pt8.rearrange("c (y z) -> c y z", z=PZ)
                        if bf16_by_tx[txc]:
                            ptv16 = get_plane16(xo + txc).rearrange(
                                "c (y z) -> c y z", z=PZ)
                        for pp, pair in enumerate(pairs_by_tx[txc]):
                            for yb in range(YB):
                                ps = pss[yb][:, :YBS * (OZ // 2)]
                                dr_mm(ps, txc, pp, pair, pt8, ptv8,
                                      yb * YBS, i == 0, i == NI - 1)
                            i += 1
                        for jj, ei in enumerate(bf16_by_tx[txc]):
                            for yb in range(YB):
                                ps = pss[yb][:, :YBS * (OZ // 2)]
                                bf16_mm(ps, txc, jj, ei, ptv16, yb * YBS,
                                        i == 0, i == NI - 1)
                            i += 1
                    for yb in range(YB):
                        y0 = yb * YBS
                        ps = pss[yb][:, :YBS * (OZ // 2)]
                        psv = ps.rearrange("c (y z) -> c y z", z=OZ // 2)
                        for q in range(2):
                            copy_out(ostv, psv, y0, q)
                    nc.sync.dma_start(
                        out=yout[:, xo * OPLANE:(xo + 1) * OPLANE],
                        in_=ostage[:, :])
                else:
                    # last plane: sequential per-yb groups so earlier yb
                    # copies and output DMAs overlap later yb matmuls
                    for yb in range(YB):
                        y0 = yb * YBS
                        ps_full = ppool.tile([128, 512], f32, name="ps")
                        ps = ps_full[:, :YBS * (OZ // 2)]
                        i = 0
                        for txc in range(5):
                            pt8 = get_plane8(xo + txc)
                            ptv8 = pt8.rearrange("c (y z) -> c y z", z=PZ)
                            if bf16_by_tx[txc]:
                                ptv16 = get_plane16(xo + txc).rearrange(
                                    "c (y z) -> c y z", z=PZ)
                            for pp, pair in enumerate(pairs_by_tx[txc]):
                                dr_mm(ps, txc, pp, pair, pt8, ptv8, y0,
                                      i == 0, i == NI - 1)
                                i += 1
                            for jj, ei in enumerate(bf16_by_tx[txc]):
                                bf16_mm(ps, txc, jj, ei, ptv16, y0,
                                        i == 0, i == NI - 1)
                                i += 1
                        psv = ps.rearrange("c (y z) -> c y z", z=OZ // 2)
                        for q in range(2):
                            copy_out(ostv, psv, y0, q)
                        nc.sync.dma_start(
                            out=yout[:, xo * OPLANE + y0 * OZ:
                                     xo * OPLANE + (y0 + YBS) * OZ],
                            in_=ostage[:, y0 * OZ:(y0 + YBS) * OZ])
    nc.finalize()
    return nc


def _prep_inputs(x, w16, w8):
    """Returns per-core in_maps. x: [2,48,48,48,64] float32."""
    in_maps = []
    for core in range(N_CORES):
        n, xs = core // 4, (core % 4) * OX
        xpadn = np.pad(x[n], ((2, 2), (2, 2), (2, 2), (0, 0)))
        slab = xpadn[xs:xs + PX]                               # [16,52,52,64]
        xc = slab.transpose(3, 0, 1, 2)                        # [64,16,52,52]
        xsh = np.zeros_like(xc)
        xsh[..., :-1] = xc[..., 1:]                            # z+1 shift
        dup = np.concatenate([xc, xsh], axis=0)                # [128,16,52,52]
        # z -> (parity, half) de-interleave for contiguous rhs slices
        dup = np.stack([dup[..., 0::2], dup[..., 1::2]], axis=3)
        dup = np.ascontiguousarray(dup).reshape(128, PX * PLANE2)
        in_maps.append({
            "xin8": np.clip(dup, -240, 240).astype(ml_dtypes.float8_e4m3fn),
            "xin16": dup.astype(ml_dtypes.bfloat16),
            "wts16": w16,
            "wts8": w8,
        })
    return in_maps


def _run(inputs, trace=False):
    x = np.asarray(inputs["x"], np.float32)
    k = _build_k(np.asarray(inputs["w000"], np.float64),
                 np.asarray(inputs["w011"], np.float64),
                 np.asarray(inputs["w101"], np.float64),
                 np.asarray(inputs["w110"], np.float64),
                 np.asarray(inputs["sc0"], np.float64),
                 np.asarray(inputs["sc1"], np.float64))
    cfg = _config(k)
    w16, w8 = _pack_weights(k, cfg)
    in_maps = _prep_inputs(x, w16, w8)

    nc = _get_nc(cfg)
    res = bass_utils.run_bass_kernel_spmd(
        nc, in_maps, core_ids=list(range(N_CORES)), trace=trace)

    inv = np.float32(1.0 / cfg[2])
    out = np.empty((2, 48, 48, 48, 64), np.float32)
    for core in range(N_CORES):
        n, xs = core // 4, (core % 4) * OX
        oc = (np.asarray(res.results[core]["yout"], dtype=np.float32)
              * inv).reshape(64, OX, OY, OZ)
        out[n, xs:xs + OX] = oc.transpose(1, 2, 3, 0)
    return out, res


def kernel(**inputs):
    out, _ = _run(inputs, trace=False)
    return out


# revision 13
# speedup vs baseline: 1.0160x; 1.0160x over previous
"""Trainium2 Bass kernel for the e3nn-style 5x5x5 SAME conv (dense_cnn).

Strategy
--------
Data-parallel: 8 shards = 2 batches x 4 x-slabs of 12 output planes each.
Each core gets a zero/halo-padded, channel-first input slab and produces
[64, 12, 48, 48].

Sparsity-aware tap packing (see baseline docstring): 57 of 125 taps are
nonzero; the conv is a sum of 39 PSUM-accumulated K=128 matmul "entries"
per output tile, using a dup-plane layout (partitions 0-63 = ch at z,
64-127 = ch at z+1, z stored de-interleaved as (parity, half)).

Mixed fp8/bf16 precision (this version): output energy is dominated by a
few entries (the center self-connection entry alone is ~69%).  The top-7
entries by energy run in bf16 as before; the remaining 32 are quantized
to fp8 e4m3 (TRN clip +-240, weights pre-scaled by a power of 2) and run
as 16 MatmulPerfMode.DoubleRow instructions, each fusing TWO entries
(K=256) at ~2x bf16 column throughput.  The DR rhs addresses the two
entries' plane slices with a hand-built 4-dim access pattern
[part][pair][y][z].  All matmuls accumulate into the same fp32 PSUM
group; the weight scale is divided out on the host after gather.  bf16
plane copies load lazily (only planes some bf16 entry reads).
Simulated end-to-end rel err: 1.82e-2 (gate 2e-2).
"""

import math

import numpy as np
import ml_dtypes
import bass_rust

import concourse.bass as bass
import concourse.mybir as mybir
from concourse import bacc, bass_utils
from concourse.tile import TileContext

MUL = 16
NB = 4
R = 2.5

N_CORES = 8
PX, PY, PZ = 16, 52, 52          # padded per-core input slab (x, y, z)
OX, OY, OZ = 12, 48, 48          # per-core output region
HPL = PZ // 2                    # 26 z-halves per parity block
PLANE2 = PY * PZ                 # 2704 cols per dup plane (y major, z=(par,h))
OPLANE = OY * OZ                 # 2304 outputs per x-plane
YB = 3                           # y-blocks of 16 rows -> N = 16*24 = 384
YBS = OY // YB
N_BF16 = 7                       # entries kept in bf16 (top by energy)


def _entries():
    """(tx, ty, c) matmul list; tap tz(s, q) = c + s - q."""
    out = []
    for tx in range(5):
        for ty in range(5):
            d2 = (tx - 2) ** 2 + (ty - 2) ** 2
            if d2 in (0, 1):
                cs = (0, 2, 4)
            elif d2 in (2, 4):
                cs = (1, 3)
            elif d2 == 5:
                cs = (2,)
            else:
                cs = ()
            for c in cs:
                out.append((tx, ty, c))
    return out


ENTRIES = _entries()
N_W = len(ENTRIES)               # 39


def _build_k(w000, w011, w101, w110, sc0, sc1):
    """Numpy port of the reference kernel build. Returns [5,5,5,64,64]."""
    s = 2
    c = np.arange(-s, s + 1.0)
    lat = np.stack(np.meshgrid(c, c, c, indexing='ij'), axis=-1)
    norm = np.linalg.norm(lat, axis=-1)
    safe = np.where(norm == 0.0, 1.0, norm)
    nvec = np.where(norm[..., None] > 0.0, lat / safe[..., None], 0.0)
    sh1 = np.sqrt(3.0) * nvec
    values = np.linspace(0.0, R, NB + 2)[1:-1]
    step = R / (NB + 1)
    d = (norm[..., None] - values) / step
    dd = np.clip(d, -1.0 + 1e-9, 1.0 - 1e-9)
    emb = np.where(np.abs(d) < 1.0,
                   1.14136 * np.e ** 2 * np.exp(-1.0 / (1.0 - dd ** 2)), 0.0)
    nlat = 125.0

    r000 = np.einsum('xyzb,buw->xyzuw', emb, w000) / nlat
    r011 = np.einsum('xyzb,buw->xyzuw', emb, w011) / nlat
    r101 = np.einsum('xyzb,buw->xyzuw', emb, w101) / nlat
    r110 = np.einsum('xyzb,buw->xyzuw', emb, w110) / nlat
    eye3 = np.eye(3)
    k00 = r000
    k01 = np.einsum('xyzuw,xyzk->xyzuwk', r011, sh1).reshape(5, 5, 5, MUL, 3 * MUL)
    k11 = np.einsum('xyzuw,ik->xyzuiwk', r101, eye3).reshape(5, 5, 5, 3 * MUL, 3 * MUL)
    k10 = np.einsum('xyzuw,xyzi->xyzuiw', r110, sh1).reshape(5, 5, 5, 3 * MUL, MUL) / np.sqrt(3.0)
    top = np.concatenate([k00, k01], axis=-1)
    bot = np.concatenate([k10, k11], axis=-1)
    k = np.concatenate([top, bot], axis=-2)

    lin00 = sc0 / np.sqrt(float(MUL))
    lin11 = np.einsum('uw,ik->uiwk', sc1 / np.sqrt(float(MUL)), eye3).reshape(3 * MUL, 3 * MUL)
    z16 = np.zeros((MUL, 3 * MUL))
    lin = np.concatenate([
        np.concatenate([lin00, z16], axis=1),
        np.concatenate([z16.T, lin11], axis=1)], axis=0)
    k[2, 2, 2] = lin
    return k


def _config(k):
    """Pick bf16 entries (top N_BF16 by output-energy) and pair the rest
    for DoubleRow, pairing within each tx group (same SBUF plane tile).

    Returns (bf16_by_tx, pairs_by_tx, wscale):
      bf16_by_tx[tx] = [entry_idx, ...]
      pairs_by_tx[tx] = [(eA, eB|None), ...]  ordered so sliceB offset >
                        sliceA offset.
    """
    Etap = np.sum(k ** 2, axis=(3, 4))
    Eent = np.zeros(N_W)
    for ei, (tx, ty, c) in enumerate(ENTRIES):
        for s in range(2):
            for q in range(2):
                tz = c + s - q
                if 0 <= tz <= 4:
                    Eent[ei] += Etap[tx, ty, tz] / 2.0
    bf16_set = set(np.argsort(-Eent)[:N_BF16].tolist())

    def soff(ei):
        _, ty, c = ENTRIES[ei]
        return ty * PZ + (c & 1) * HPL + (c >> 1)

    bf16_by_tx, pairs_by_tx = [], []
    max8 = 0.0
    for tx in range(5):
        idxs = [ei for ei, e in enumerate(ENTRIES) if e[0] == tx]
        bf16_by_tx.append([ei for ei in idxs if ei in bf16_set])
        fp8 = sorted((ei for ei in idxs if ei not in bf16_set), key=soff)
        pr = []
        for j in range(0, len(fp8) - 1, 2):
            pr.append((fp8[j], fp8[j + 1]))
        if len(fp8) % 2:
            pr.append((fp8[-1], None))
        pairs_by_tx.append(pr)
        for ei in fp8:
            tx_, ty, c = ENTRIES[ei]
            for s in range(2):
                for q in range(2):
                    tz = c + s - q
                    if 0 <= tz <= 4:
                        max8 = max(max8, np.abs(k[tx_, ty, tz]).max())
    wscale = 2.0 ** math.floor(math.log2(224.0 / max8))
    return bf16_by_tx, pairs_by_tx, wscale


def _entry_block(k, ei, scale):
    """[128,128] f32 block for entry ei: B[s*64+ci, q*64+co] =
    k[tx,ty,c+s-q][ci,co] * scale."""
    tx, ty, c = ENTRIES[ei]
    B = np.zeros((128, 128), np.float32)
    for s in range(2):
        for q in range(2):
            tz = c + s - q
            if 0 <= tz <= 4:
                B[s * 64:(s + 1) * 64, q * 64:(q + 1) * 64] = \
                    k[tx, ty, tz] * scale
    return B


def _pack_weights(k, cfg):
    bf16_by_tx, pairs_by_tx, wscale = cfg
    w16 = np.concatenate(
        [_entry_block(k, ei, wscale)
         for tx in range(5) for ei in bf16_by_tx[tx]], axis=1)
    blocks8 = []
    for tx in range(5):
        for (eA, eB) in pairs_by_tx[tx]:
            blocks8.append(_entry_block(k, eA, wscale))
            blocks8.append(_entry_block(k, eB, wscale)
                           if eB is not None else np.zeros((128, 128), np.float32))
    w8 = np.concatenate(blocks8, axis=1)
    w16 = np.ascontiguousarray(w16).astype(ml_dtypes.bfloat16)
    w8 = np.clip(np.ascontiguousarray(w8), -240, 240) \
        .astype(ml_dtypes.float8_e4m3fn)
    return w16, w8


_NC = None
_NC_KEY = None


def _get_nc(cfg):
    global _NC, _NC_KEY
    key = (tuple(map(tuple, cfg[0])), tuple(map(tuple, cfg[1])))
    if _NC is None or _NC_KEY != key:
        _NC = _build_nc(cfg)
        _NC_KEY = key
    return _NC


def _build_nc(cfg):
    bf16_by_tx, pairs_by_tx, _ = cfg
    n16 = sum(len(b) for b in bf16_by_tx)
    npr = sum(len(p) for p in pairs_by_tx)
    NI = n16 + npr                   # accumulation-group length per y-block

    nc = bacc.Bacc("TRN2", target_bir_lowering=False)
    f32 = mybir.dt.float32
    bf16 = mybir.dt.bfloat16
    fp8 = mybir.dt.float8e4

    xin8 = nc.dram_tensor("xin8", [128, PX * PLANE2], fp8, kind="ExternalInput")
    xin16 = nc.dram_tensor("xin16", [128, PX * PLANE2], bf16,
                           kind="ExternalInput")
    wts16 = nc.dram_tensor("wts16", [128, max(n16, 1) * 128], bf16,
                           kind="ExternalInput")
    wts8 = nc.dram_tensor("wts8", [128, npr * 256], fp8, kind="ExternalInput")
    # output staged/DMAed as bf16 (halves output DMA; ~1e-3 extra rel err,
    # negligible in quadrature); host converts back to f32 and unscales
    yout = nc.dram_tensor("yout", [64, OX * OPLANE], bf16,
                          kind="ExternalOutput")

    with TileContext(nc) as tc:
        with tc.tile_pool(name="wpool", bufs=1) as wpool, \
             tc.tile_pool(name="xpool8", bufs=8) as xpool8, \
             tc.tile_pool(name="xpool16", bufs=8) as xpool16, \
             tc.tile_pool(name="opool", bufs=2) as opool, \
             tc.tile_pool(name="ppool", bufs=6, space="PSUM") as ppool:

            planes8, planes16 = {}, {}

            def _load_plane(px, pool, dt, src, tag):
                # dup plane tile: col (y, par, h); partitions 0-63 hold ch
                # at z=2h+par, 64-127 hold z+1.  One DMA per plane: each
                # dynamic-DMA issue costs ~650ns on the Sync queue, and the
                # 3-y-block entry group needs the whole plane anyway.
                pt = pool.tile([128, PLANE2], dt, tag=tag, name=tag)
                base = px * PLANE2
                nc.sync.dma_start(out=pt[:, :],
                                  in_=src[:, base:base + PLANE2])
                return pt

            def get_plane8(px):
                if px not in planes8:
                    planes8[px] = _load_plane(px, xpool8, fp8, xin8,
                                              "plane8")
                return planes8[px]

            def get_plane16(px):
                # lazy: only planes actually read by a bf16 entry load
                if px not in planes16:
                    planes16[px] = _load_plane(px, xpool16, bf16, xin16,
                                               "plane16")
                return planes16[px]

            # per-tx weight chunks, interleaved with plane loads so the
            # first matmuls' dependencies land first
            n16_tx = [len(b) for b in bf16_by_tx]
            npr_tx = [len(p) for p in pairs_by_tx]
            off16 = [sum(n16_tx[:t]) for t in range(5)]
            off8 = [sum(npr_tx[:t]) for t in range(5)]
            wt16_chunks = [None] * 5
            wt8_chunks = [None] * 5

            def load_chunk(txc):
                if npr_tx[txc]:
                    w8c = wpool.tile([128, npr_tx[txc] * 256], fp8,
                                     tag="wt8", bufs=5, name="wt8")
                    o = off8[txc] * 256
                    nc.sync.dma_start(out=w8c[:, :],
                                      in_=wts8[:, o:o + npr_tx[txc] * 256])
                    wt8_chunks[txc] = w8c
                if n16_tx[txc]:
                    w16c = wpool.tile([128, n16_tx[txc] * 128], bf16,
                                      tag="wt16", bufs=5, name="wt16")
                    o = off16[txc] * 128
                    nc.sync.dma_start(out=w16c[:, :],
                                      in_=wts16[:, o:o + n16_tx[txc] * 128])
                    wt16_chunks[txc] = w16c

            # prefetch in first-consumption order: the xo=0 instruction
            # stream needs (fp8 plane tx, wt chunk tx, bf16 plane tx if
            # that tx has bf16 entries) for tx = 0..4.  Plane 0 issues
            # first: its transfer (346KB) is the first matmul's critical
            # path, while chunk 0 (65KB) fits in its issue shadow.
            get_plane8(0)
            load_chunk(0)
            if n16_tx[0]:
                get_plane16(0)
            for txc in range(1, 5):
                load_chunk(txc)
                get_plane8(txc)
                if n16_tx[txc]:
                    get_plane16(txc)
            get_plane8(5)

            # warm-up: dummy matmuls ramp the PE clock (0.65->2.4 GHz after
            # ~3us busy) while the first input DMAs are in flight
            warm = wpool.tile([128, 128], bf16, tag="warm", bufs=1,
                              name="warm")
            nc.any.memset(warm[:, :], 0)
            wps = ppool.tile([128, 512], f32, tag="wps", bufs=1, name="wps")
            for _ in range(16):
                nc.tensor.matmul(wps[:, :128], warm[:, :], warm[:, :],
                                 start=True, stop=True)

            def zoff_of(ei):
                _, _, c = ENTRIES[ei]
                return (c & 1) * HPL + (c >> 1)

            def bf16_mm(ps, txc, jj, ei, ptv16, y0, start, stop):
                _, ty, _ = ENTRIES[ei]
                zo = zoff_of(ei)
                lhsT = wt16_chunks[txc][:, jj * 128:(jj + 1) * 128]
                rhs = ptv16[:, y0 + ty:y0 + ty + YBS, zo:zo + OZ // 2]
                nc.tensor.matmul(ps[:, :], lhsT, rhs, start=start, stop=stop)

            def dr_mm(ps, txc, pp, pair, pt8, ptv8, y0, start, stop):
                eA, eB = pair
                _, tyA, _ = ENTRIES[eA]
                zA = zoff_of(eA)
                sA = ptv8[:, y0 + tyA:y0 + tyA + YBS, zA:zA + OZ // 2]
                if eB is None:
                    delta = 1
                else:
                    _, tyB, _ = ENTRIES[eB]
                    delta = (tyB - tyA) * PZ + (zoff_of(eB) - zA)
                rhs = sA.unsqueeze(1)
                pat = [list(p) for p in rhs.ap]
                pat[1] = [delta, 2]
                rhs.ap = bass_rust.VecI64Pair(pat)
                lhsT = wt8_chunks[txc][:, :].rearrange(
                    "c (p j m) -> c p j m", j=2, m=128)[:, pp, :, :]
                nc.tensor.matmul(ps[:, :], lhsT, rhs,
                                 perf_mode=mybir.MatmulPerfMode.DoubleRow,
                                 start=start, stop=stop)

            def copy_out(ostv, psv, y0, q):
                # alternate DVE / ACT so the two per-bank copies drain in
                # parallel instead of serializing on Vector
                dst = ostv[:, y0:y0 + YBS, q:OZ:2]
                src = psv[q * 64:(q + 1) * 64, :, :]
                if q == 0:
                    nc.vector.tensor_copy(dst, src)
                else:
                    nc.scalar.activation(
                        dst, src, mybir.ActivationFunctionType.Copy)

            for xo in range(OX):
                ostage = opool.tile([64, OPLANE], bf16, name="ostage")
                ostv = ostage.rearrange("c (y z) -> c y z", z=OZ)
                last = xo == OX - 1
                if not last:
                    pss = [ppool.tile([128, 512], f32, name="ps")
                           for _ in range(YB)]
                    i = 0
                    for txc in range(5):
                        pt8 = get_plane8(xo + txc)
                        ptv8 = pt8.rearrange("c (y z) -> c y z", z=PZ)
                        if bf16_by_tx[txc]:
                            ptv16 = get_plane16(xo + txc).rearrange(
                                "c (y z) -> c y z", z=PZ)
                        for pp, pair in enumerate(pairs_by_tx[txc]):
                            for yb in range(YB):
                                ps = pss[yb][:, :YBS * (OZ // 2)]
                                dr_mm(ps, txc, pp, pair, pt8, ptv8,
                                      yb * YBS, i == 0, i == NI - 1)
                            i += 1
                        for jj, ei in enumerate(bf16_by_tx[txc]):
                            for yb in range(YB):
                                ps = pss[yb][:, :YBS * (OZ // 2)]
                                bf16_mm(ps, txc, jj, ei, ptv16, yb * YBS,
                                        i == 0, i == NI - 1)
                            i += 1
                    for yb in range(YB):
                        y0 = yb * YBS
                        ps = pss[yb][:, :YBS * (OZ // 2)]
                        psv = ps.rearrange("c (y z) -> c y z", z=OZ // 2)
                        for q in range(2):
                            copy_out(ostv, psv, y0, q)
                    nc.sync.dma_start(
                        out=yout[:, xo * OPLANE:(xo + 1) * OPLANE],
                        in_=ostage[:, :])
                else:
                    # last plane: sequential per-yb groups so earlier yb
                    # copies and output DMAs overlap later yb matmuls
                    for yb in range(YB):
                        y0 = yb * YBS
                        ps_full = ppool.tile([128, 512], f32, name="ps")
                        ps = ps_full[:, :YBS * (OZ // 2)]
                        i = 0
                        for txc in range(5):
                            pt8 = get_plane8(xo + txc)
                            ptv8 = pt8.rearrange("c (y z) -> c y z", z=PZ)
                            if bf16_by_tx[txc]:
                                ptv16 = get_plane16(xo + txc).rearrange(
                                    "c (y z) -> c y z", z=PZ)
                            for pp, pair in enumerate(pairs_by_tx[txc]):
                                dr_mm(ps, txc, pp, pair, pt8, ptv8, y0,
                                      i == 0, i == NI - 1)
                                i += 1
                            for jj, ei in enumerate(bf16_by_tx[txc]):
                                bf16_mm(ps, txc, jj, ei, ptv16, y0,
                                        i == 0, i == NI - 1)
                                i += 1
                        psv = ps.rearrange("c (y z) -> c y z", z=OZ // 2)
                        for q in range(2):
                            copy_out(ostv, psv, y0, q)
                        nc.sync.dma_start(
                            out=yout[:, xo * OPLANE + y0 * OZ:
                                     xo * OPLANE + (y0 + YBS) * OZ],
                            in_=ostage[:, y0 * OZ:(y0 + YBS) * OZ])
    nc.finalize()
    return nc


def _prep_inputs(x, w16, w8):
    """Returns per-core in_maps. x: [2,48,48,48,64] float32."""
    in_maps = []
    for core in range(N_CORES):
        n, xs = core // 4, (core % 4) * OX
        xpadn = np.pad(x[n], ((2, 2), (2, 2), (2, 2), (0, 0)))
        slab = xpadn[xs:xs + PX]                               # [16,52,52,64]
        xc = slab.transpose(3, 0, 1, 2)                        # [64,16,52,52]
        xsh = np.zeros_like(xc)
        xsh[..., :-1] = xc[..., 1:]                            # z+1 shift
        dup = np.concatenate([xc, xsh], axis=0)                # [128,16,52,52]
        # z -> (parity, half) de-interleave for contiguous rhs slices
        dup = np.stack([dup[..., 0::2], dup[..., 1::2]], axis=3)
        dup = np.ascontiguousarray(dup).reshape(128, PX * PLANE2)
        in_maps.append({
            "xin8": np.clip(dup, -240, 240).astype(ml_dtypes.float8_e4m3fn),
            "xin16": dup.astype(ml_dtypes.bfloat16),
            "wts16": w16,
            "wts8": w8,
        })
    return in_maps


def _run(inputs, trace=False):
    x = np.asarray(inputs["x"], np.float32)
    k = _build_k(np.asarray(inputs["w000"], np.float64),
                 np.asarray(inputs["w011"], np.float64),
                 np.asarray(inputs["w101"], np.float64),
                 np.asarray(inputs["w110"], np.float64),
                 np.asarray(inputs["sc0"], np.float64),
                 np.asarray(inputs["sc1"], np.float64))
    cfg = _config(k)
    w16, w8 = _pack_weights(k, cfg)
    in_maps = _prep_inputs(x, w16, w8)

    nc = _get_nc(cfg)
    res = bass_utils.run_bass_kernel_spmd(
        nc, in_maps, core_ids=list(range(N_CORES)), trace=trace)

    inv = np.float32(1.0 / cfg[2])
    out = np.empty((2, 48, 48, 48, 64), np.float32)
    for core in range(N_CORES):
        n, xs = core // 4, (core % 4) * OX
        oc = (np.asarray(res.results[core]["yout"], dtype=np.float32)
              * inv).reshape(64, OX, OY, OZ)
        out[n, xs:xs + OX] = oc.transpose(1, 2, 3, 0)
    return out, res


def kernel(**inputs):
    out, _ = _run(inputs, trace=False)
    return out
